# revision 7
# baseline (speedup 1.0000x reference)
"""Trainium2 Bass kernel for nn_ComponentAwa (landmark-gated CNN).

Data-parallel over batch: 32 images -> 8 cores x 4 images.

Per-core pipeline:
  Phase 1 (towers): 68 independent 5-layer depthwise 3x3 conv towers on
    landmarks. Each (channel, layer) = 3 f32r matmuls with banded (Toeplitz)
    stationaries contracting over H, accumulating the 3 W-shifts in PSUM.
    Layer-5 matmuls accumulate whole channel groups directly into 4
    persistent "gate" PSUM banks (eye/nose/jaw/mouth sums for free).
  Phase 2 (dense): 4 branches x 3 layers of dense 16->16 3x3 conv on
    features, in an H-strip layout [(hl 8, ci 16)=128 partitions,
    (strip 22, w 130)]; M=(r 6, co 16)=96; 3 dw-shift matmuls per layer.
    Inter-layer evacuation is a contiguous partition-shift(+16) ACT copy
    with per-partition bias. Branch L3 output is gated:
    out += (psum + bias) * gate  via fused scalar_tensor_tensor on DVE,
    with invalid strip rows self-masked by zero gate values.
"""

import sys

sys.path.insert(0, "/opt/trn_rl_repo")

import numpy as np

import concourse.bass as bass  # noqa: F401
import concourse.tile as tile
from concourse import bacc, mybir
from concourse.bass_utils import run_bass_kernel_spmd

F32 = mybir.dt.float32
F32R = mybir.dt.float32r
AF = mybir.ActivationFunctionType
ALU = mybir.AluOpType

N_CORES = 8
B = 32              # full batch
BI = B // N_CORES   # images per core = 4
L = 68              # landmark channels
C = 16              # feature channels
H = W = 128
WPAD = W + 2        # zero-padded width
S = 22              # h-strips of 6 (stride 6, 8-row window with halo)
NL = 5              # tower layers
GROUPS = [           # channel lists per gate
    list(range(17, 27)) + list(range(36, 48)),   # eye
    list(range(27, 36)),                         # nose
    list(range(0, 17)),                          # jaw
    list(range(48, 68)),                         # mouth
]

_CACHE = {}


def _build_nc():
    nc = bacc.Bacc("TRN2", target_bir_lowering=False, debug=False,
                   num_devices=N_CORES)

    lm_in = nc.dram_tensor("lm_in", [BI, L, H, W], F32R, kind="ExternalInput")
    fe_in = nc.dram_tensor("fe_in", [BI, C, H, W], F32R, kind="ExternalInput")
    bv_in = nc.dram_tensor("bv_in", [L, NL, 3, 128, 128], F32R,
                           kind="ExternalInput")
    af_in = nc.dram_tensor("af_in", [12, 3, 128, 128], F32R,
                           kind="ExternalInput")
    zero_in = nc.dram_tensor("zero_in", [128, WPAD], F32R, kind="ExternalInput")
    btow_in = nc.dram_tensor("btow_in", [128, L * NL], F32,
                             kind="ExternalInput")  # tower biases (replicated)
    bg_in = nc.dram_tensor("bg_in", [128, 4], F32, kind="ExternalInput")
    bfe_in = nc.dram_tensor("bfe_in", [96, 12], F32, kind="ExternalInput")
    bfe128_in = nc.dram_tensor("bfe128_in", [128, 12], F32, kind="ExternalInput")
    out_dram = nc.dram_tensor("out", [BI, C, H, W], F32, kind="ExternalOutput")

    with tile.TileContext(nc) as tc:
        with (
            tc.tile_pool(name="drp", bufs=1, space="DRAM") as drp,
            tc.tile_pool(name="wsb", bufs=1) as wsb,        # resident weights
            tc.tile_pool(name="bvp", bufs=2) as bvp,        # tower stationaries
            tc.tile_pool(name="xtp", bufs=6) as xtp,        # tower act tiles
            tc.tile_pool(name="gsb", bufs=4) as gsb,        # gates in SBUF
            tc.tile_pool(name="x0p", bufs=2) as x0p,        # dense inputs
            tc.tile_pool(name="xap", bufs=2) as xap,        # dense L1 out
            tc.tile_pool(name="xbp", bufs=2) as xbp,        # dense L2 out
            tc.tile_pool(name="grp", bufs=2) as grp,        # replicated gates
            tc.tile_pool(name="tmp", bufs=2) as tmp,        # gating temp
            tc.tile_pool(name="oap", bufs=2) as oap,        # out accumulators
        ):
            gate_dram = drp.tile([4, BI, H, W], F32)
            # ------- resident small weights -------
            aft = wsb.tile([128, 12, 3, 128], F32R)
            nc.sync.dma_start(aft[:], af_in.ap().rearrange("t d h m -> h t d m"))
            btow = wsb.tile([128, L * NL], F32)
            nc.sync.dma_start(btow[:], btow_in.ap())
            bgt = wsb.tile([128, 4], F32)
            nc.sync.dma_start(bgt[:], bg_in.ap())
            bfet = wsb.tile([96, 12], F32)
            nc.sync.dma_start(bfet[:], bfe_in.ap())
            bfe128t = wsb.tile([128, 12], F32)
            nc.sync.dma_start(bfe128t[:], bfe128_in.ap())

            # channel -> gate index
            c2g = {}
            for g, cl in enumerate(GROUPS):
                for c in cl:
                    c2g[c] = g
            # first/last channel per gate (in processing order 0..67)
            first_c = {g: min(cl) for g, cl in enumerate(GROUPS)}
            g_first = [min(cl) for cl in GROUPS]
            g_last = [max(cl) for cl in GROUPS]

            with (
                tc.tile_pool(name="psg", bufs=4, space="PSUM") as psg,
                tc.tile_pool(name="psw", bufs=3, space="PSUM") as psw,
            ):
                gate_ps = [psg.tile([128, BI, W], F32, tag="gate", name=f"gate_ps{g_}") for g_ in range(4)]

                # =================== Phase 1: towers ===================
                for c in range(L):
                    bvt = bvp.tile([128, NL, 3, 128], F32R, tag="bv")
                    nc.sync.dma_start(
                        bvt[:], bv_in.ap()[c].rearrange("l d h m -> h l d m"))
                    xt = xtp.tile([128, BI, WPAD], F32R, tag="xt")
                    nc.gpsimd.memset(xt[:, :, 0::W + 1].bitcast(F32), 0.0)
                    nc.sync.dma_start(
                        xt[:, :, 1:W + 1],
                        lm_in.ap().rearrange("b c h w -> c h b w")[c])

                    for layer in range(NL):
                        if layer < NL - 1:
                            ps = psw.tile([128, BI, W], F32, tag="tw")
                            for dw in range(3):
                                nc.tensor.matmul(
                                    ps[:],
                                    bvt[:, layer, dw, :],
                                    xt[:, :, dw:dw + W],
                                    start=(dw == 0), stop=(dw == 2),
                                )
                            nxt = xtp.tile([128, BI, WPAD], F32R, tag="xt")
                            nc.gpsimd.memset(
                                nxt[:, :, 0::W + 1].bitcast(F32), 0.0)
                            bidx = layer * L + c
                            nc.scalar.activation(
                                nxt[:, :, 1:W + 1], ps[:], AF.Identity,
                                bias=btow[:, bidx:bidx + 1], scale=1.0)
                            xt = nxt
                        else:
                            g = c2g[c]
                            for dw in range(3):
                                nc.tensor.matmul(
                                    gate_ps[g][:],
                                    bvt[:, layer, dw, :],
                                    xt[:, :, dw:dw + W],
                                    start=(c == g_first[g] and dw == 0),
                                    stop=(c == g_last[g] and dw == 2),
                                )

                # gate evac: PSUM -> SBUF (+group bias) -> DRAM bounce
                for g in range(4):
                    gt = gsb.tile([128, BI, W], F32, tag="gate_sb")
                    nc.scalar.activation(gt[:], gate_ps[g][:], AF.Identity,
                                         bias=bgt[:, g:g + 1], scale=1.0)
                    nc.sync.dma_start(
                        gate_dram[g].rearrange("b h w -> h b w"), gt[:])

            # =================== Phase 2: dense branches ===================
            with tc.tile_pool(name="psd", bufs=8, space="PSUM") as psd:
                # s-chunks of the 22 strips, PSUM-bank-sized (<=512 cols)
                chunks = [(0, 4), (4, 4), (8, 4), (12, 4), (16, 4), (20, 2)]

                def pads_pre(t):
                    # zero w-pad cols (never written later)
                    nc.gpsimd.memset(t[:, :, 0::W + 1].bitcast(F32), 0.0)

                def pads_post(t):
                    # zero invalid strip rows: (s=0, hl=0) and (s=21, hl>=3)
                    nc.gpsimd.memset(t[0:16, 0:1, :].bitcast(F32), 0.0)
                    nc.sync.dma_start(t[48:128, S - 1:S, :],
                                      zero_in.ap()[48:128, :].unsqueeze(1))

                def halo_fixups(t):
                    # hl=0 of strip s  <- hl'=6 (parts 96:112) of strip s-1
                    nc.sync.dma_start(t[0:16, 1:S, 1:W + 1],
                                      t[96:112, 0:S - 1, 1:W + 1])
                    # hl=7 of strip s  <- hl'=1 (parts 16:32) of strip s+1
                    nc.sync.dma_start(t[112:128, 0:S - 1, 1:W + 1],
                                      t[16:32, 1:S, 1:W + 1])

                def load_x0(img):
                    x0 = x0p.tile([128, S, WPAD], F32R, tag="x0")
                    pads_pre(x0)
                    pads_post(x0)
                    for hl in range(8):
                        s_lo = 0 if hl >= 1 else 1
                        s_hi = min(S - 1, (H - hl) // 6)
                        n_s = s_hi - s_lo + 1
                        src = fe_in.ap()[img][:, (6 * s_lo + hl - 1)::6, :][:, :n_s, :]
                        nc.sync.dma_start(
                            x0[hl * 16:(hl + 1) * 16, s_lo:s_hi + 1, 1:W + 1],
                            src)
                    return x0

                def conv_layer(xin, widx, psum_tiles):
                    # 3 dw-shift matmuls per chunk; dw outer for lhsT reuse
                    for dw in range(3):
                        for ci_, (c0, cn) in enumerate(chunks):
                            nc.tensor.matmul(
                                psum_tiles[ci_][:, :cn * W],
                                aft[:, widx, dw, :],
                                xin[:, c0:c0 + cn, dw:dw + W],
                                start=(dw == 0), stop=(dw == 2),
                            )

                def evac_layer(psum_tiles, xout, widx):
                    bias = bfe128t[:, widx:widx + 1]
                    for ci_, (c0, cn) in enumerate(chunks):
                        psv = psum_tiles[ci_][:, :cn * W].rearrange(
                            "q (s w) -> q s w", w=W)
                        nc.scalar.activation(
                            xout[:, c0:c0 + cn, 1:W + 1], psv, AF.Identity,
                            bias=bias, scale=1.0)
                    halo_fixups(xout)
                    pads_post(xout)

                for img in range(BI):
                    x0 = load_x0(img)
                    oacc = oap.tile([128, S, W], F32, tag="oacc")
                    for g in range(4):
                        # gate replicated to all 8 strip rows x 16 co:
                        # gr[(hl,co), s, w] = gate[6s+hl-1, w]
                        gr = grp.tile([128, S, W], F32, tag="grep")
                        nc.gpsimd.memset(gr[0:16, 0:1, :], 0.0)
                        nc.sync.dma_start(gr[48:128, S - 1:S, :],
                                          zero_in.ap().bitcast(F32)[48:128, 0:W].unsqueeze(1))
                        for hl in range(8):
                            s_lo = 0 if hl >= 1 else 1
                            s_hi = min(S - 1, (H - hl) // 6)
                            n_s = s_hi - s_lo + 1
                            src = gate_dram[g, img][(6 * s_lo + hl - 1)::6, :][:n_s, :]
                            src_b = src.unsqueeze(0).broadcast_to([16, n_s, W])
                            nc.gpsimd.dma_start(
                                gr[hl * 16:(hl + 1) * 16, s_lo:s_hi + 1, :], src_b)

                        xa = xap.tile([128, S, WPAD], F32R, tag="xa")
                        pads_pre(xa)
                        ps1 = [psd.tile([128, 4 * W], F32, tag="pd", name=f"ps1_{i_}") for i_ in range(len(chunks))]
                        conv_layer(x0, g * 3 + 0, ps1)
                        evac_layer(ps1, xa, g * 3 + 0)

                        xb = xbp.tile([128, S, WPAD], F32R, tag="xb")
                        pads_pre(xb)
                        ps2 = [psd.tile([128, 4 * W], F32, tag="pd", name=f"ps2_{i_}") for i_ in range(len(chunks))]
                        conv_layer(xa, g * 3 + 1, ps2)
                        evac_layer(ps2, xb, g * 3 + 1)

                        ps3 = [psd.tile([128, 4 * W], F32, tag="pd", name=f"ps3_{i_}") for i_ in range(len(chunks))]
                        conv_layer(xb, g * 3 + 2, ps3)
                        # gated accumulate: oacc (+)= (psum + bias) * gate
                        bias3 = bfe128t[:, g * 3 + 2:g * 3 + 3]
                        for ci_, (c0, cn) in enumerate(chunks):
                            psv = ps3[ci_][:, :cn * W].rearrange(
                                "q (s w) -> q s w", w=W)
                            if g == 0:
                                nc.vector.scalar_tensor_tensor(
                                    oacc[:, c0:c0 + cn, :], psv, bias3,
                                    gr[:, c0:c0 + cn, :],
                                    op0=ALU.add, op1=ALU.mult)
                            else:
                                tt = tmp.tile([128, 4, W], F32, tag="tmp")
                                nc.vector.scalar_tensor_tensor(
                                    tt[:, :cn, :], psv, bias3,
                                    gr[:, c0:c0 + cn, :],
                                    op0=ALU.add, op1=ALU.mult)
                                nc.vector.tensor_add(
                                    oacc[:, c0:c0 + cn, :],
                                    oacc[:, c0:c0 + cn, :], tt[:, :cn, :])

                    # unpack interior strip rows -> DRAM out[img, co, 6s+r, w]
                    for r in range(6):
                        n_s = len(range(r, H, 6))
                        nc.sync.dma_start(
                            out_dram.ap()[img][:, r::6, :][:, :n_s, :],
                            oacc[(r + 1) * 16:(r + 2) * 16, :n_s, :])

    nc.compile()
    return nc


def _host_weights(w_lm, b_lm, w_fea, b_fea):
    w_lm = np.asarray(w_lm, np.float32)
    b_lm = np.asarray(b_lm, np.float32)
    w_fea = np.asarray(w_fea, np.float32)
    b_fea = np.asarray(b_fea, np.float32)

    # tower banded stationaries Bv[c, l, dw][h, hp] = w_lm[l, c, 0, h-hp+1, dw]
    bv = np.zeros((L, NL, 3, 128, 128), np.float32)
    i = np.arange(128)
    for d in range(3):
        hp = i[(i + d - 1 >= 0) & (i + d - 1 < 128)]
        h = hp + d - 1
        # bv[c, l, dw, h, hp] = w_lm[l, c, 0, d, dw]
        bv[:, :, :, h, hp] = w_lm[:, :, 0, d, :].transpose(1, 0, 2)[:, :, :, None]

    # dense stationaries A[(g l), dw][(hl ci), (hl' co)] = w_fea[g,l,co,ci,hl-hl'+1,dw]
    af = np.zeros((12, 3, 128, 128), np.float32)
    for g in range(4):
        for layer in range(3):
            for dh in range(3):
                for hlp in range(8):
                    hl = hlp + dh - 1
                    if 0 <= hl < 8:
                        af[g * 3 + layer, :, hl * 16:hl * 16 + 16,
                           hlp * 16:hlp * 16 + 16] = (
                            w_fea[g, layer, :, :, dh, :]  # [co, ci, dw]
                            .transpose(2, 1, 0))           # [dw, ci, co]

    btow = np.zeros((128, L * NL), np.float32)
    for layer in range(NL - 1):
        btow[:, layer * L:(layer + 1) * L] = b_lm[layer][None, :]

    bg = np.zeros((128, 4), np.float32)
    for g, cl in enumerate(GROUPS):
        bg[:, g] = b_lm[NL - 1, cl].sum()

    bfe = np.zeros((96, 12), np.float32)
    bfe128 = np.zeros((128, 12), np.float32)
    for g in range(4):
        for layer in range(3):
            bfe[:, g * 3 + layer] = np.tile(b_fea[g, layer], 6)
            bfe128[:, g * 3 + layer] = np.tile(b_fea[g, layer], 8)

    return bv, af, btow, bg, bfe, bfe128


def kernel(**inputs):
    landmarks = np.ascontiguousarray(np.asarray(inputs["landmarks"], np.float32))
    features = np.ascontiguousarray(np.asarray(inputs["features"], np.float32))

    if "nc" not in _CACHE:
        _CACHE["nc"] = _build_nc()
    nc = _CACHE["nc"]

    bv, af, btow, bg, bfe, bfe128 = _host_weights(
        inputs["w_lm"], inputs["b_lm"], inputs["w_fea"], inputs["b_fea"])
    zeros = np.zeros((128, WPAD), np.float32)

    in_maps = []
    for i in range(N_CORES):
        sl = slice(i * BI, (i + 1) * BI)
        in_maps.append({
            "lm_in": landmarks[sl],
            "fe_in": features[sl],
            "bv_in": bv,
            "af_in": af,
            "btow_in": btow,
            "bg_in": bg,
            "bfe_in": bfe,
            "bfe128_in": bfe128,
            "zero_in": zeros,
        })

    res = run_bass_kernel_spmd(nc, in_maps, core_ids=list(range(N_CORES)))
    out = np.concatenate([r["out"] for r in res.results], axis=0)
    return out.astype(np.float32)
